# revision 1
# baseline (speedup 1.0000x reference)
"""Trainium2 Bass kernel for nn_CircumpunctAttention_17136919511703.

Sharding: 16 heads tensor-parallel over 8 cores (2 heads/core); W_out
row-parallel with the cross-core partial sum done on the host.

Math simplifications (validated vs the jax reference):
  - attn = softmax(s)*ap*ex renormalized by (sum + 1e-8): the aperture
    gate ap cancels exactly up to the 1e-8 term, so it is dropped.
    converged = (e' @ v) / (e' @ 1) with e'_st = ex_s * exp(scoresT_st)
    (sender gate folded into vpp).
  - no softmax max-subtraction (scores are bounded, |s| < 4).
  - the per-head "aperture chamber" (valve in/out, phase rotation, chi)
    is a per-head linear map on the head dim -> folded into W_out on the
    host: W'_h = c_h * Wout_h @ R_h.

Schedule (engines execute strictly in program order, so emission order
is the schedule):
  wave(half): i/o/v/ex projections for one T-half, 4 concurrent PSUM
    accumulation streams paced to the xT tile DMAs; ex is computed in
    natural [t, h] orientation (tiny free dim) so no transposes or M=2
    waves are needed; v transposed to natural and gated by ex into vpp.
  attention: 4 j-blocks of 512 t-cols; per block a software-pipelined
    i-loop emits scores(i+2)+exp(i+2) BEFORE PT(i) so the PE never
    waits on the ACT exp; pt accumulators [65, 512] live in one 2-bank
    PSUM slot.  post(j) normalizes via DVE reciprocal + Pool
    partition-broadcast + DVE multiply (no PE, no ACT).  Final output
    matmuls for block j are interleaved into block j+1's loop; y tiles
    staged via Pool-engine copies and DMA'd out per 128-row slice.
PSUM budget: tags sc(2x[128,1024]) + pt(1x[128,1024]) + py(2x[128,512])
  = exactly 8 banks, shared by all phases.
"""

import math
import os
from contextlib import ExitStack

import numpy as np

import concourse.bass as bass
import concourse.mybir as mybir
import concourse.tile as tile
from concourse import bacc
from concourse.bass_utils import run_bass_kernel_spmd
from concourse.masks import make_identity

T, D, H, DH = 2048, 1024, 16, 64
NCORES = 8
HPC = H // NCORES          # heads per core = 2
KW = HPC * DH              # per-core head width = 128
SCALE = math.sqrt(DH)
DT = D // 128              # d tiles = 8
TT = T // 128              # t/s tiles = 16
F32 = mybir.dt.float32
F32R = mybir.dt.float32r
BF16 = mybir.dt.bfloat16
AF = mybir.ActivationFunctionType

_CACHE = {}
LAST_RESULTS = None


def _build_nc():
    nc = bacc.Bacc()
    xT = nc.declare_dram_parameter("xT", [128, DT, T], BF16, isOutput=False)
    wiT = nc.declare_dram_parameter("wiT", [128, DT, KW], BF16, isOutput=False)
    woT = nc.declare_dram_parameter("woT", [128, DT, KW], BF16, isOutput=False)
    wvT = nc.declare_dram_parameter("wvT", [128, DT, KW], BF16, isOutput=False)
    weT = nc.declare_dram_parameter("weT", [128, DT, HPC], BF16, isOutput=False)
    webn = nc.declare_dram_parameter("webn", [128, TT * HPC], F32, isOutput=False)
    wpT = nc.declare_dram_parameter("wpT", [KW, D], F32R, isOutput=False)
    y = nc.declare_dram_parameter("y", [T, D], BF16, isOutput=True)

    with tile.TileContext(nc) as tc, ExitStack() as ctx:
        _body(ctx, tc, xT, wiT, woT, wvT, weT, webn, wpT, y)
    nc.compile()
    return nc


def _body(ctx, tc, xT, wiT, woT, wvT, weT, webn, wpT, y):
    nc = tc.nc
    P = 128
    HB = 512                   # j-block width
    NJ = T // HB               # 4 j-blocks

    const = ctx.enter_context(tc.tile_pool(name="const", bufs=1))
    persist = ctx.enter_context(tc.tile_pool(name="persist", bufs=1))
    eTp = ctx.enter_context(tc.tile_pool(name="eTp", bufs=4))
    stage = ctx.enter_context(tc.tile_pool(name="stage", bufs=2))
    psum = ctx.enter_context(tc.tile_pool(name="psum", bufs=1, space="PSUM"))

    # ---- DMA issue order == consumption order ----
    wiT_sb = const.tile([P, DT, KW], BF16)
    woT_sb = const.tile([P, DT, KW], BF16)
    wvT_sb = const.tile([P, DT, KW], BF16)
    weT_sb = const.tile([P, DT, HPC], BF16)
    xT_sb = persist.tile([P, DT, T], BF16)
    wpT_sb = const.tile([KW, D], F32R)
    webn_sb = const.tile([P, TT * HPC], F32)
    nc.sync.dma_start(out=wiT_sb, in_=wiT[:, :, :])
    nc.sync.dma_start(out=xT_sb[:, 0, 0:1024], in_=xT[:, 0, 0:1024])
    nc.sync.dma_start(out=woT_sb, in_=woT[:, :, :])
    nc.sync.dma_start(out=wvT_sb, in_=wvT[:, :, :])
    nc.sync.dma_start(out=weT_sb, in_=weT[:, :, :])
    for a in range(1, DT):
        nc.sync.dma_start(out=xT_sb[:, a, 0:1024], in_=xT[:, a, 0:1024])
    nc.sync.dma_start(out=webn_sb, in_=webn[:, :])
    for a in range(DT):
        nc.sync.dma_start(out=xT_sb[:, a, 1024:T], in_=xT[:, a, 1024:T])
    nc.sync.dma_start(out=wpT_sb, in_=wpT[:, :])

    ident = const.tile([P, P], F32)
    make_identity(nc, ident)
    onep = const.tile([P, 1], F32)
    nc.vector.memset(onep, 1.0)

    innerT = persist.tile([P, T], F32R)
    outerT = persist.tile([P, T], F32R)
    vT = persist.tile([P, T], F32)
    vpp = persist.tile([P, TT, HPC, 65], F32R)
    # col 64 must be 1.0 (ones row for the E sums); cols 0:64 are fully
    # overwritten by the v copies, so a contiguous whole-tile memset works
    nc.vector.memset(vpp.bitcast(F32), 1.0)
    exz = persist.tile([P, TT * HPC], F32)   # sigmoid(ex logits), [t, i*2+h]
    convT = persist.tile([KW, T], F32R)

    def xchunk(a, c):
        # c indexes 512-wide chunks of T
        return xT_sb[:, a, c * 512:(c + 1) * 512]

    def wave(half):
        psI = psum.tile([P, 1024], F32, tag="sc", bufs=2, name="psI")
        psO = psum.tile([P, 1024], F32, tag="sc", bufs=2, name="psO")
        psV = psum.tile([P, 1024], F32, tag="pt", bufs=1, name="psV")
        for a in range(DT):
            st, sp = (a == 0), (a == DT - 1)
            for j2 in range(2):
                c = half * 2 + j2
                nc.tensor.matmul(
                    psI[:, j2 * 512:(j2 + 1) * 512],
                    lhsT=wiT_sb[:, a, :], rhs=xchunk(a, c),
                    start=st, stop=sp)
                nc.tensor.matmul(
                    psO[:, j2 * 512:(j2 + 1) * 512],
                    lhsT=woT_sb[:, a, :], rhs=xchunk(a, c),
                    start=st, stop=sp)
                nc.tensor.matmul(
                    psV[:, j2 * 512:(j2 + 1) * 512],
                    lhsT=wvT_sb[:, a, :], rhs=xchunk(a, c),
                    start=st, stop=sp)
        hs = slice(half * 1024, (half + 1) * 1024)
        nc.vector.tensor_copy(out=innerT[:, hs], in_=psI)
        nc.vector.tensor_copy(out=outerT[:, hs], in_=psO)
        nc.scalar.copy(out=vT[:, hs], in_=psV)

        # ex logits per t-tile (natural [t, h] layout, all tiny); one
        # accumulation group per PSUM bank (start=True resets bank-wide)
        for k in range(8):
            m = half * 8 + k
            pex = psum.tile([P, HPC], F32, tag="py", bufs=2, name="pex")
            for a in range(DT):
                nc.tensor.matmul(
                    pex,
                    lhsT=xT_sb[:, a, m * P:(m + 1) * P],
                    rhs=weT_sb[:, a, :],
                    start=(a == 0), stop=(a == DT - 1))
            nc.vector.tensor_add(out=exz[:, HPC * m:HPC * (m + 1)], in0=pex,
                                 in1=webn_sb[:, HPC * m:HPC * (m + 1)])
        cs = slice(half * 16, (half + 1) * 16)
        nc.scalar.activation(out=exz[:, cs], in_=exz[:, cs],
                             func=AF.Exp, scale=-1.0)
        nc.vector.tensor_scalar(out=exz[:, cs], in0=exz[:, cs],
                                scalar1=onep, scalar2=None,
                                op0=mybir.AluOpType.add)
        nc.vector.reciprocal(out=exz[:, cs], in_=exz[:, cs])

        # v -> natural per-head layout, gated by ex
        pstr = psum.tile([P, 1024], F32, tag="sc", bufs=2, name="pstr")
        for k in range(8):
            i = half * 8 + k
            nc.tensor.transpose(
                pstr[:, k * P:(k + 1) * P], vT[:, i * P:(i + 1) * P], ident)
        psr = pstr.rearrange("p (k c) -> p k c", c=P)
        g8 = slice(half * 8, (half + 1) * 8)
        for h in range(HPC):
            nc.vector.tensor_copy(
                out=vpp[:, g8, h, 0:DH],
                in_=psr[:, :, h * DH:(h + 1) * DH])
        for k in range(8):
            i = half * 8 + k
            for h in range(HPC):
                nc.vector.tensor_scalar_mul(
                    out=vpp[:, i, h, :], in0=vpp[:, i, h, :],
                    scalar1=exz[:, HPC * i + h:HPC * i + h + 1])

    wave(0)
    wave(1)

    # ---- attention: one flat software-pipelined loop over k = j*16+i ----
    NK = NJ * TT               # 64

    def sc_exp(k):
        i, j = k % TT, k // TT
        sc = psum.tile([P, 1024], F32, tag="sc", bufs=2, name="sc")
        nc.tensor.matmul(
            sc[:, 0:512],
            lhsT=outerT[0:DH, i * P:(i + 1) * P].bitcast(F32R),
            rhs=innerT[0:DH, j * HB:(j + 1) * HB].bitcast(F32R),
            start=True, stop=True)
        nc.tensor.matmul(
            sc[:, 512:1024],
            lhsT=outerT[DH:KW, i * P:(i + 1) * P].bitcast(F32R),
            rhs=innerT[DH:KW, j * HB:(j + 1) * HB].bitcast(F32R),
            start=True, stop=True)
        eT = eTp.tile([P, 1024], F32R, tag="e", name="eT")
        nc.scalar.activation(out=eT, in_=sc, func=AF.Exp, scale=1.0)
        return eT

    def emit_final(m, tail=False):
        # y partial for t-tile m; PSUM->SBUF copies on DVE (ACT helps in the
        # tail, once the exp stream is done), then DMA
        y_sb = stage.tile([P, D], BF16, tag="y", name="y_sb")
        for o in range(2):
            py = psum.tile([P, 512], F32, tag="py", bufs=2, name="py")
            nc.tensor.matmul(
                py,
                lhsT=convT[:, m * P:(m + 1) * P].bitcast(F32R),
                rhs=wpT_sb[:, o * 512:(o + 1) * 512].bitcast(F32R),
                start=True, stop=True)
            if tail and o == 0:
                nc.scalar.copy(out=y_sb[:, 0:512], in_=py)
            else:
                nc.vector.tensor_copy(out=y_sb[:, o * 512:(o + 1) * 512],
                                      in_=py)
            nc.sync.dma_start(out=y[m * P:(m + 1) * P, o * 512:(o + 1) * 512],
                              in_=y_sb[:, o * 512:(o + 1) * 512])

    def post(j, pt, tail=False):
        # conv = PT[0:64] / E, E in row 64; no PE, no ACT.  One fast DVE
        # copy frees the pt PSUM slot; the rest runs from SBUF off the
        # critical path.  In the tail the slot is never reused, so read
        # the PSUM directly and skip the copy.
        if tail:
            pt_sb = pt
        else:
            pt_sb = stage.tile([65, 1024], F32, tag="ptsb", name="pt_sb")
            nc.vector.tensor_copy(out=pt_sb, in_=pt)
        f_row = stage.tile([1, 1024], F32R, tag="f", name="f_row")
        with nc.allow_low_precision(reason="f32r is a 4-byte container"):
            nc.vector.reciprocal(out=f_row, in_=pt_sb[64:65, 0:1024])
        fbc = stage.tile([DH, 1024], F32R, tag="fb", name="fbc")
        nc.gpsimd.partition_broadcast(fbc, f_row, channels=DH)
        jc = slice(j * HB, (j + 1) * HB)
        nc.vector.tensor_mul(out=convT[0:DH, jc], in0=pt_sb[0:DH, 0:512],
                             in1=fbc[:, 0:512])
        nc.vector.tensor_mul(out=convT[DH:KW, jc], in0=pt_sb[0:DH, 512:1024],
                             in1=fbc[:, 512:1024])

    y_pend = []                # finals not yet emitted: list of m
    pt = None
    eTs = {0: sc_exp(0), 1: sc_exp(1)}
    for k in range(NK):
        i, j = k % TT, k // TT
        if k + 2 < NK:
            eTs[k + 2] = sc_exp(k + 2)
        if i == 0:
            pt = psum.tile([65, 1024], F32, tag="pt", bufs=1, name="pt")
        if i in (4, 7, 10, 13) and y_pend:
            emit_final(y_pend.pop(0))
        eT = eTs.pop(k)
        nc.tensor.matmul(
            pt[0:65, 0:512],
            lhsT=vpp[:, i, 0, :].bitcast(F32R),
            rhs=eT[:, 0:512],
            start=(i == 0), stop=(i == TT - 1))
        nc.tensor.matmul(
            pt[0:65, 512:1024],
            lhsT=vpp[:, i, 1, :].bitcast(F32R),
            rhs=eT[:, 512:1024],
            start=(i == 0), stop=(i == TT - 1))
        if i == TT - 1:
            post(j, pt, tail=(j == NJ - 1))
            y_pend.extend(range(j * 4, (j + 1) * 4))

    for m in y_pend:
        emit_final(m, tail=True)


def _sigmoid(z):
    return 1.0 / (1.0 + np.exp(-z))


def _prep_in_maps(inputs):
    x = np.ascontiguousarray(np.asarray(inputs["x"], np.float32)[0])  # [T, D]
    xT = np.ascontiguousarray(x.T)                                    # [D, T]
    Wi = np.asarray(inputs["Wi_w"], np.float32).reshape(H, DH, D) / SCALE
    Wo = np.asarray(inputs["Wo_w"], np.float32).reshape(H, DH, D)
    Wv = np.asarray(inputs["Wv_w"], np.float32).reshape(H, DH, D)
    We = np.asarray(inputs["We_w"], np.float32)                       # [H, D]
    We_b = np.asarray(inputs["We_b"], np.float32)                     # [H]
    Wout = np.asarray(inputs["Wout_w"], np.float32)                   # [D, D]
    beta = np.asarray(inputs["beta"], np.float32)
    iv = np.asarray(inputs["iv"], np.float32)
    ov = np.asarray(inputs["ov"], np.float32)
    chi = np.asarray(inputs["chi"], np.float32)

    # chamber folded into Wout: W'_h = c_h * Wout_h @ R_h
    ang = np.float32(math.pi) * _sigmoid(beta)
    c_h = _sigmoid(iv) * _sigmoid(ov) * np.tanh(chi)                  # [H]
    cos_a, sin_a = np.cos(ang), np.sin(ang)
    HALF = DH // 2
    Wp = np.zeros((H, D, DH), np.float32)
    for h in range(H):
        Wh = Wout[:, h * DH:(h + 1) * DH]
        Wp[h][:, :HALF] = c_h[h] * (Wh[:, :HALF] * cos_a[h] + Wh[:, HALF:] * sin_a[h])
        Wp[h][:, HALF:] = c_h[h] * (-Wh[:, :HALF] * sin_a[h] + Wh[:, HALF:] * cos_a[h])

    def dtile(arr):  # [D, X] -> [128, DT, X] (d-tile-major, partition-contig)
        return np.ascontiguousarray(
            arr.reshape(DT, 128, arr.shape[1]).transpose(1, 0, 2))

    import ml_dtypes
    bf16 = ml_dtypes.bfloat16
    xTr = dtile(xT).astype(bf16)
    in_maps = []
    for c in range(NCORES):
        hs = slice(HPC * c, HPC * (c + 1))
        wiT = dtile(Wi[hs].reshape(KW, D).T).astype(bf16)
        woT = dtile(Wo[hs].reshape(KW, D).T).astype(bf16)
        wvT = dtile(Wv[hs].reshape(KW, D).T).astype(bf16)
        weT = dtile(We[hs].T).astype(bf16)
        webn = np.ascontiguousarray(np.broadcast_to(
            np.tile(We_b[hs], TT), (128, TT * HPC)).astype(np.float32))
        wpT = np.ascontiguousarray(
            Wp[hs].transpose(0, 2, 1).reshape(KW, D))                 # [128, D]
        in_maps.append(dict(xT=xTr, wiT=wiT, woT=woT, wvT=wvT,
                            weT=weT, webn=webn, wpT=wpT))
    return in_maps


def kernel(**inputs):
    global LAST_RESULTS
    if "nc" not in _CACHE:
        _CACHE["nc"] = _build_nc()
    nc = _CACHE["nc"]
    in_maps = _prep_in_maps(inputs)
    trace = os.environ.get("CIRC_TRACE", "") not in ("", "0")
    res = run_bass_kernel_spmd(
        nc, in_maps, core_ids=list(range(NCORES)), trace=trace)
    LAST_RESULTS = res
    y = res.results[0]["y"].astype(np.float32)
    for c in range(1, NCORES):
        y = y + res.results[c]["y"].astype(np.float32)
    return y.reshape(1, T, D)

